# revision 15
# baseline (speedup 1.0000x reference)
"""Trainium2 Bass kernel for nn_MultiHeadSelfAttentionBlock.

Strategy (data-parallel over batch, B=32 -> 4 per core on 8 cores):
  - BN folded to per-channel scale/shift, written into a zero-padded [C,34,34]
    SBUF buffer (borders = conv padding).
  - q 1x1 conv as 40 accumulating matmuls; results stored c-major:
    qbuf[s%128, c*8 + t] so each head's Q^T tile is one contiguous slice
    (the torch .view head-split bug resolves to l = 16*c + 2*t + par, k = s_lo).
  - depthwise 3x3/s2 conv + BN + 1x1 proj: im2col tap windows staged once per
    channel chunk (shared by k and v), then 9 taps x 5 chunks of accumulating
    matmuls with weights W_tapT[c,kd] = wT[c,kd]*bnscale[c]*dw[c,tap]
    (k-side prescaled by 1/sqrt(64)); BN shift folded into a constant.
    k and v share one PSUM tile via PE column groups (0,0) / (0,64).
  - logits computed transposed [p, l] (lhsT = kf duplicated into both
    partition halves so K=64 matmul pairs pack into PE row groups 0/64);
    softmax denominator comes free as row 64 of the o-matmul by appending a
    ones column to V^T.
  - o = V'^T @ exp(logitsT) in PSUM [65, l]; denominator rows extracted by
    ACT into partitions 0/32, reciprocal via ACT ln -> exp(-x), broadcast
    across partitions via a DRAM bounce, and the normalize multiply fused
    into the PSUM->SBUF move (DVE STT), writing o_resh[(n,vd), s] directly
    via a scatter AP.
  - output proj as accumulating matmuls; layer-scale applied via a
    precomputed [128,1024] mask during the PSUM->SBUF move; residual added on
    GPSIMD; fp32 everywhere, matmuls use float32r (fast fp32 weight path).
"""

from contextlib import ExitStack

import os

import numpy as np

import concourse.bacc as bacc
import concourse.bass as bass
import concourse.tile as tile
from concourse import mybir
from concourse.masks import make_identity

F32 = mybir.dt.float32
F32R = mybir.dt.float32r
ALU = mybir.AluOpType
ACTF = mybir.ActivationFunctionType

B, C, H, W = 32, 640, 32, 32
NH, KD, VD = 8, 64, 64
S = H * W            # 1024
P = 256              # key/value positions (16x16)
EPS = 1e-3
N_CORES = 8
BPC = B // N_CORES   # 4 batch items per core
NCH = C // 128       # 5 channel chunks


def _r(ap):
    return ap.bitcast(F32R)


def _fap(base, free_off, dims):
    """AP with base's partition dim and explicit free dims [[step, count],...]."""
    return bass.AP(tensor=base.tensor, offset=base.offset + free_off,
                   ap=[base.ap[0]] + dims)


def build_nc():
    nc = bacc.Bacc(None, target_bir_lowering=False, debug=False)

    din = {}
    def dt_in(name, shape):
        din[name] = nc.dram_tensor(name, shape, F32, kind="ExternalInput")
        return din[name]

    x4 = dt_in("x", [BPC, C, H, W])
    q_w = dt_in("q_w", [NH * KD, C])
    k_w = dt_in("k_w", [KD, C])
    v_w = dt_in("v_w", [VD, C])
    out_w = dt_in("out_w", [C, NH * VD])
    k_dw = dt_in("k_dw_w", [C, 1, 3, 3])
    v_dw = dt_in("v_dw_w", [C, 1, 3, 3])
    for p in ("in", "k", "v"):
        for s in ("gamma", "beta", "mean", "var"):
            dt_in(f"{p}_bn_{s}", [C])
    ls = dt_in("ls_gamma", [W])
    out4 = nc.dram_tensor("out", [BPC, C, H, W], F32, kind="ExternalOutput")
    KSTAGE = int(os.environ.get("KSTAGE", "99"))

    with tile.TileContext(nc) as tc, ExitStack() as ctx:
        wp = ctx.enter_context(tc.tile_pool(name="wp", bufs=1))
        stg = ctx.enter_context(tc.tile_pool(name="stg", bufs=2))
        # PSUM pools (bank-granular): mm 2 + lg 2 + op 4 = 8 banks
        mmp = ctx.enter_context(tc.tile_pool(name="mmp", bufs=2, space="PSUM"))
        lgp = ctx.enter_context(tc.tile_pool(name="lgp", bufs=2, space="PSUM"))
        opp = ctx.enter_context(tc.tile_pool(name="opp", bufs=4, space="PSUM"))
        # SBUF working pools
        xin = ctx.enter_context(tc.tile_pool(name="xin", bufs=3))
        xres = ctx.enter_context(tc.tile_pool(name="xres", bufs=2))
        xnp = ctx.enter_context(tc.tile_pool(name="xnp", bufs=NCH))
        xcp = ctx.enter_context(tc.tile_pool(name="xcp", bufs=2))
        qbp = ctx.enter_context(tc.tile_pool(name="qbp", bufs=1))
        ep = ctx.enter_context(tc.tile_pool(name="ep", bufs=4))
        dal = ctx.enter_context(tc.tile_pool(name="dal", bufs=2))
        rbcp = ctx.enter_context(tc.tile_pool(name="rbcp", bufs=2))
        orp = ctx.enter_context(tc.tile_pool(name="orp", bufs=4))
        osb = ctx.enter_context(tc.tile_pool(name="osb", bufs=2))
        kvp = ctx.enter_context(tc.tile_pool(name="kvp", bufs=2))
        drp = ctx.enter_context(tc.tile_pool(name="drp", bufs=4, space="DRAM"))

        # ---------------- setup: identity ----------------
        ident = wp.tile([128, 128], F32, tag="ident", name="ident")
        make_identity(nc, ident[:])

        def pe_transpose(dst_sbuf_ap, src_sbuf_ap, scale=1.0, rnd=False):
            """dst[f, p] = src[p, f] via PE; src [p, f] with p,f <= 128."""
            pdim = src_sbuf_ap.shape[0]
            fdim = src_sbuf_ap.free_size()
            tp = mmp.tile([128, 512], F32, tag="mm", name="tp")
            nc.tensor.transpose(tp[:fdim, :pdim], src_sbuf_ap,
                                ident[:pdim, :pdim])
            dst = _r(dst_sbuf_ap) if rnd else dst_sbuf_ap
            nc.scalar.activation(dst, tp[:fdim, :pdim], ACTF.Copy,
                                 scale=scale)

        # ---------------- setup: BN scale/shift ----------------
        eps_t = wp.tile([128, 1], F32, tag="eps", name="eps")
        nc.gpsimd.memset(eps_t[:], EPS)
        bnss = {}  # (prefix, ch) -> (scale [128,1], shift [128,1])
        for pfx in ("in", "k", "v"):
            for ch in range(NCH):
                g = stg.tile([128, 1], F32, tag="bnl0", name="bnl0")
                be = stg.tile([128, 1], F32, tag="bnl1", name="bnl1")
                m = stg.tile([128, 1], F32, tag="bnl2", name="bnl2")
                v = stg.tile([128, 1], F32, tag="bnl3", name="bnl3")
                cs = slice(128 * ch, 128 * (ch + 1))
                nc.sync.dma_start(out=g[:], in_=din[f"{pfx}_bn_gamma"][cs].unsqueeze(1))
                nc.sync.dma_start(out=be[:], in_=din[f"{pfx}_bn_beta"][cs].unsqueeze(1))
                nc.sync.dma_start(out=m[:], in_=din[f"{pfx}_bn_mean"][cs].unsqueeze(1))
                nc.sync.dma_start(out=v[:], in_=din[f"{pfx}_bn_var"][cs].unsqueeze(1))
                sc = wp.tile([128, 1], F32, tag=f"sc_{pfx}{ch}", name=f"sc_{pfx}{ch}")
                sh = wp.tile([128, 1], F32, tag=f"sh_{pfx}{ch}", name=f"sh_{pfx}{ch}")
                nc.scalar.activation(sc[:], v[:], ACTF.Sqrt, bias=eps_t[:])
                nc.vector.reciprocal(sc[:], sc[:])
                nc.vector.tensor_mul(sc[:], sc[:], g[:])
                nc.vector.tensor_mul(sh[:], m[:], sc[:])
                nc.vector.tensor_sub(sh[:], be[:], sh[:])
                bnss[(pfx, ch)] = (sc, sh)

        # ---------------- setup: transposed weights ----------------
        q_wT = [wp.tile([128, 512], F32, tag=f"qwT{j}", name=f"qwT{j}")
                for j in range(NCH)]
        for i in range(4):
            st = stg.tile([128, 640], F32, tag="wstage", name="wstage")
            nc.sync.dma_start(out=st[:], in_=q_w[128 * i:128 * (i + 1), :])
            for j in range(NCH):
                pe_transpose(q_wT[j][:, 128 * i:128 * (i + 1)],
                             st[:, 128 * j:128 * (j + 1)], rnd=True)

        out_wT = [wp.tile([128, 640], F32, tag=f"owT{j}", name=f"owT{j}")
                  for j in range(4)]
        for i in range(NCH):
            st = stg.tile([128, 512], F32, tag="wstage", name="wstage")
            nc.sync.dma_start(out=st[:], in_=out_w[128 * i:128 * (i + 1), :])
            for j in range(4):
                pe_transpose(out_wT[j][:, 128 * i:128 * (i + 1)],
                             st[:, 128 * j:128 * (j + 1)], rnd=True)

        kv_wT = {}
        for nm, wdr, scl in (("k", k_w, 0.125), ("v", v_w, 1.0)):
            st = stg.tile([64, 640], F32, tag="kvstage", name="kvstage")
            nc.sync.dma_start(out=st[:], in_=wdr[:, :])
            for j in range(NCH):
                wt = wp.tile([128, 64], F32, tag=f"{nm}wT{j}", name=f"{nm}wT{j}")
                pe_transpose(wt[:], st[:, 128 * j:128 * (j + 1)], scale=scl)
                kv_wT[(nm, j)] = wt

        # ---------------- setup: conv tap weights + consts ----------------
        wtap = {}
        kv_const = {}
        for nm, dwdr in (("k", k_dw), ("v", v_dw)):
            cps = mmp.tile([64, 512], F32, tag="mm", name="cps")
            for ch in range(NCH):
                dw = stg.tile([128, 9], F32, tag="dwl", name="dwl")
                nc.sync.dma_start(
                    out=dw[:],
                    in_=dwdr[128 * ch:128 * (ch + 1), 0, :, :].rearrange(
                        "c a b -> c (a b)"))
                sc, sh = bnss[(nm, ch)]
                s9 = stg.tile([128, 9], F32, tag="s9", name="s9")
                nc.vector.tensor_scalar_mul(s9[:], dw[:], sc[:])
                for t in range(9):
                    wtt = wp.tile([128, 64], F32, tag=f"wtap_{nm}{ch}_{t}",
                                  name=f"wtap_{nm}{ch}_{t}")
                    nc.vector.tensor_scalar_mul(_r(wtt[:]),
                                                kv_wT[(nm, ch)][:],
                                                s9[:, t:t + 1])
                    wtap[(nm, ch, t)] = wtt
                nc.tensor.matmul(cps[:64, 0:1], kv_wT[(nm, ch)][:], sh[:],
                                 start=(ch == 0), stop=(ch == NCH - 1))
            cst = wp.tile([64, 1], F32, tag=f"const_{nm}", name=f"const_{nm}")
            nc.scalar.activation(cst[:], cps[:64, 0:1], ACTF.Copy)
            kv_const[nm] = cst

        # ---------------- setup: zero/one consts ----------------
        zeros16 = wp.tile([128, 16], F32, tag="zeros16", name="zeros16")
        nc.gpsimd.memset(zeros16[:], 0.0)
        ones1 = wp.tile([128, 1], F32, tag="ones1", name="ones1")
        nc.gpsimd.memset(ones1[:], 1.0)

        # ---------------- setup: layer-scale mask ----------------
        lsmask = wp.tile([128, 1024], F32, tag="lsmask", name="lsmask")
        ls_b = bass.AP(tensor=ls, offset=0, ap=[[0, 128], [1, 32]])
        for rr in range(32):
            nc.gpsimd.dma_start(out=lsmask[:, 32 * rr:32 * (rr + 1)], in_=ls_b)

        # ================= per batch item =================
        im2col_eng = [nc.vector, nc.gpsimd, nc.scalar]
        for b in range(BPC):
            # ---- load x, BN into flat xn buffer ----
            xns = []
            for ch in range(NCH):
                xt = xin.tile([128, 1024], F32, tag="xin", name="xin")
                nc.sync.dma_start(
                    out=xt[:],
                    in_=x4[b, 128 * ch:128 * (ch + 1), :, :].rearrange(
                        "c h w -> c (h w)"))
                xn = xnp.tile([128, 1024], F32, tag="xn", name="xn")
                sc, sh = bnss[("in", ch)]
                nc.vector.tensor_scalar(
                    out=_r(xn[:]), in0=xt[:],
                    scalar1=sc[:], scalar2=sh[:], op0=ALU.mult, op1=ALU.add)
                xns.append(xn)

            # ---- q projection -> qbuf [s%128, c*8 + t] (c-major) ----
            qbuf = qbp.tile([128, 4096], F32, tag="qbuf", name="qbuf")
            for t in range(8):
                qp = mmp.tile([128, 512], F32, tag="mm", name="qp")
                for ch in range(NCH):
                    lhsT = xns[ch][:, 128 * t:128 * (t + 1)]
                    nc.tensor.matmul(qp[:], _r(lhsT), _r(q_wT[ch][:]),
                                     start=(ch == 0), stop=(ch == NCH - 1))
                nc.vector.tensor_copy(_r(_fap(qbuf[:], t, [[8, 512]])), qp[:])

            if KSTAGE == 1:
                nc.sync.dma_start(
                    out=out4[b, 0:128, :, :].rearrange("c h w -> c (h w)"),
                    in_=qbuf[:, 0:1024])
                continue
            # ---- im2col + dw-conv + BN + 1x1 proj for k and v ----
            kfp = mmp.tile([64, 256], F32, tag="mm", name="kfp")
            vfp = mmp.tile([64, 256], F32, tag="mm", name="vfp")
            for ch in range(NCH):
                xc = xcp.tile([128, 9 * 256], F32, tag="xcol", name="xcol")
                xnv = xns[ch][:].rearrange("p (a b) -> p a b", a=32)
                for t in range(9):
                    dy, dx = t // 3, t % 3
                    oh0 = 1 if dy == 0 else 0
                    ow0 = 1 if dx == 0 else 0
                    if oh0:
                        nc.vector.tensor_copy(_r(xc[:, 256 * t:256 * t + 16]),
                                              zeros16[:])
                    if ow0:
                        nc.vector.tensor_copy(
                            _r(_fap(xc[:], 256 * t, [[16, 16], [1, 1]])),
                            zeros16[:])
                    r0 = 2 * oh0 + dy - 1
                    c0 = 2 * ow0 + dx - 1
                    srcap = xnv[:, r0:r0 + 2 * (16 - oh0) - 1:2,
                                c0:c0 + 2 * (16 - ow0) - 1:2]
                    dst2 = _r(_fap(xc[:], 256 * t + 16 * oh0 + ow0,
                                   [[16, 16 - oh0], [1, 16 - ow0]]))
                    if t % 3 == 2:
                        nc.scalar.activation(dst2, srcap, ACTF.Copy)
                    else:
                        im2col_eng[t % 3].tensor_copy(dst2, srcap)
                for t in range(9):
                    first = (ch == 0 and t == 0)
                    last = (ch == NCH - 1 and t == 8)
                    xslice = xc[:, 256 * t:256 * (t + 1)]
                    nc.tensor.matmul(
                        kfp[:], _r(wtap[("k", ch, t)][:]), _r(xslice),
                        start=first, stop=last)
                    nc.tensor.matmul(
                        vfp[:], _r(wtap[("v", ch, t)][:]), _r(xslice),
                        start=first, stop=last)
            # kf duplicated into both halves (base-partition match for logits)
            kfdup = kvp.tile([128, 256], F32, tag="f_k", name="f_k")
            nc.vector.tensor_scalar_add(_r(kfdup[0:64, :]), kfp[:],
                                        kv_const["k"][:])
            nc.vector.tensor_scalar_add(_r(kfdup[64:128, :]), kfp[:],
                                        kv_const["k"][:])
            vf = kvp.tile([64, 256], F32, tag="f_v", name="f_v")
            nc.vector.tensor_scalar_add(vf[:], vfp[:],
                                        kv_const["v"][:])

            # V' = vf^T with ones column: 2 tiles [128, 65]
            vT = []
            for pt in range(2):
                vpt = kvp.tile([128, 65], F32, tag=f"vT{pt}", name=f"vT{pt}")
                pe_transpose(vpt[:, 0:64], vf[:, 128 * pt:128 * (pt + 1)],
                             rnd=True)
                nc.vector.tensor_copy(_r(vpt[:, 64:65]), ones1[:])
                vT.append(vpt)

            if KSTAGE == 2:
                nc.sync.dma_start(
                    out=out4[b, 0:128, 0:8, :].rearrange("c h w -> c (h w)"),
                    in_=kfdup[:, :])
                nc.sync.dma_start(
                    out=out4[b, 128:256, 0:2, :].rearrange("c h w -> c (h w)"),
                    in_=vT[0][:, 0:64])
                continue
            o_resh = [orp.tile([128, 1024], F32, tag="oresh", name="oresh")
                      for _ in range(4)]

            # ---- attention heads (pairs share a reciprocal) ----
            for pair in range(4):
                dall = dal.tile([33, 1024], F32, tag="dall", name="dall")
                nc.gpsimd.memset(dall[:], 1.0)
                ops_pair = []
                for n in (2 * pair, 2 * pair + 1):
                    E = [ep.tile([128, 1024], F32, tag="E", name="E")
                         for _ in range(2)]
                    for pt in range(2):
                        for par in range(2):
                            lg = lgp.tile([128, 512], F32, tag="lg", name="lg")
                            rhs = qbuf[64 * par:64 * (par + 1),
                                       512 * n:512 * (n + 1)]
                            nc.tensor.matmul(
                                lg[:],
                                _r(kfdup[64 * par:64 * (par + 1),
                                         128 * pt:128 * (pt + 1)]),
                                _r(rhs), start=True, stop=True)
                            nc.scalar.activation(
                                _r(E[pt][:, 512 * par:512 * (par + 1)]), lg[:],
                                ACTF.Exp)
                    o_ps = []
                    for par in range(2):
                        op_t = opp.tile([65, 512], F32, tag="op", name="op")
                        for pt in range(2):
                            nc.tensor.matmul(
                                op_t[:], _r(vT[pt][:]),
                                _r(E[pt][:, 512 * par:512 * (par + 1)]),
                                start=(pt == 0), stop=(pt == 1))
                        nc.scalar.activation(
                            dall[32 * (n % 2):32 * (n % 2) + 1,
                                 512 * par:512 * (par + 1)],
                            op_t[64:65, :], ACTF.Copy)
                        o_ps.append(op_t)
                    ops_pair.append((n, o_ps))

                rec = dal.tile([33, 1024], F32, tag="rec", name="rec")
                nc.scalar.activation(rec[:], dall[:], ACTF.Ln)
                nc.scalar.activation(rec[:], rec[:], ACTF.Exp, scale=-1.0)

                for n, o_ps in ops_pair:
                    dsc = drp.tile([1, 1024], F32, tag="dscr", name="dscr")
                    nc.gpsimd.dma_start(
                        out=dsc[:], in_=rec[32 * (n % 2):32 * (n % 2) + 1, :])
                    rbc = rbcp.tile([64, 1024], F32, tag="rbc", name="rbc")
                    nc.gpsimd.dma_start(
                        out=rbc[:],
                        in_=bass.AP(tensor=dsc.tensor, offset=dsc.offset,
                                    ap=[[0, 64], [1, 1024]]))
                    dst = o_resh[n // 2]
                    for par in range(2):
                        # scatter: col = 16*c + 2*t + par, iteration c-major
                        out_ap = _fap(dst[64 * (n % 2):64 * (n % 2) + 64], par,
                                      [[16, 64], [2, 8]])
                        nc.vector.scalar_tensor_tensor(
                            out=_r(out_ap), in0=o_ps[par][0:64, :], scalar=1.0,
                            in1=rbc[:, 512 * par:512 * (par + 1)],
                            op0=ALU.mult, op1=ALU.mult)

            if KSTAGE == 3:
                for c2 in range(4):
                    nc.sync.dma_start(
                        out=out4[b, 128 * c2:128 * (c2 + 1), :, :].rearrange(
                            "c h w -> c (h w)"),
                        in_=o_resh[c2][:, :])
                continue
            # ---- output projection + layer scale + residual ----
            for ch in range(NCH):
                xr = xres.tile([128, 1024], F32, tag="xres", name="xres")
                nc.sync.dma_start(
                    out=xr[:],
                    in_=x4[b, 128 * ch:128 * (ch + 1), :, :].rearrange(
                        "c h w -> c (h w)"))
                ot = osb.tile([128, 1024], F32, tag="outsb", name="outsb")
                for shalf in range(2):
                    po = mmp.tile([128, 512], F32, tag="mm", name="po")
                    for nv in range(4):
                        nc.tensor.matmul(
                            po[:],
                            _r(out_wT[nv][:, 128 * ch:128 * (ch + 1)]),
                            _r(o_resh[nv][:, 512 * shalf:512 * (shalf + 1)]),
                            start=(nv == 0), stop=(nv == 3))
                    sl = slice(512 * shalf, 512 * (shalf + 1))
                    nc.vector.scalar_tensor_tensor(
                        out=ot[:, sl], in0=po[:], scalar=1.0,
                        in1=lsmask[:, sl], op0=ALU.mult, op1=ALU.mult)
                    nc.gpsimd.tensor_tensor(
                        out=ot[:, sl], in0=ot[:, sl], in1=xr[:, sl], op=ALU.add)
                nc.sync.dma_start(
                    out=out4[b, 128 * ch:128 * (ch + 1), :, :].rearrange(
                        "c h w -> c (h w)"),
                    in_=ot[:])

    nc.finalize()
    return nc


_NC_CACHE = None


def kernel(**inputs):
    global _NC_CACHE
    from concourse.bass_utils import run_bass_kernel_spmd

    if _NC_CACHE is None:
        _NC_CACHE = build_nc()
    nc = _NC_CACHE

    x = np.ascontiguousarray(np.asarray(inputs["x"], dtype=np.float32))
    wnames = ["q_w", "k_w", "v_w", "out_w", "k_dw_w", "v_dw_w", "ls_gamma"] + \
        [f"{p}_bn_{s}" for p in ("in", "k", "v")
         for s in ("gamma", "beta", "mean", "var")]
    base = {n: np.ascontiguousarray(np.asarray(inputs[n], dtype=np.float32))
            for n in wnames}
    in_maps = []
    for c in range(N_CORES):
        m = dict(base)
        m["x"] = x[c * BPC:(c + 1) * BPC]
        in_maps.append(m)

    res = run_bass_kernel_spmd(nc, in_maps, core_ids=list(range(N_CORES)))
    out = np.concatenate([res.results[c]["out"] for c in range(N_CORES)], axis=0)
    return out.astype(np.float32)


# revision 16
# speedup vs baseline: 1.0570x; 1.0570x over previous
"""Trainium2 Bass kernel for nn_MultiHeadSelfAttentionBlock.

Strategy (data-parallel over batch, B=32 -> 4 per core on 8 cores):
  - BN folded to per-channel scale/shift, written into a zero-padded [C,34,34]
    SBUF buffer (borders = conv padding).
  - q 1x1 conv as 40 accumulating matmuls; results stored c-major:
    qbuf[s%128, c*8 + t] so each head's Q^T tile is one contiguous slice
    (the torch .view head-split bug resolves to l = 16*c + 2*t + par, k = s_lo).
  - depthwise 3x3/s2 conv + BN + 1x1 proj: im2col tap windows staged once per
    channel chunk (shared by k and v), then 9 taps x 5 chunks of accumulating
    matmuls with weights W_tapT[c,kd] = wT[c,kd]*bnscale[c]*dw[c,tap]
    (k-side prescaled by 1/sqrt(64)); BN shift folded into a constant.
    k and v share one PSUM tile via PE column groups (0,0) / (0,64).
  - logits computed transposed [p, l] (lhsT = kf duplicated into both
    partition halves so K=64 matmul pairs pack into PE row groups 0/64);
    softmax denominator comes free as row 64 of the o-matmul by appending a
    ones column to V^T.
  - o = V'^T @ exp(logitsT) in PSUM [65, l]; denominator rows extracted by
    ACT into partitions 0/32, reciprocal via ACT ln -> exp(-x), broadcast
    across partitions via a DRAM bounce, and the normalize multiply fused
    into the PSUM->SBUF move (DVE STT), writing o_resh[(n,vd), s] directly
    via a scatter AP.
  - output proj as accumulating matmuls; layer-scale applied via a
    precomputed [128,1024] mask during the PSUM->SBUF move; residual added on
    GPSIMD; fp32 everywhere, matmuls use float32r (fast fp32 weight path).
"""

from contextlib import ExitStack

import os

import numpy as np

import concourse.bacc as bacc
import concourse.bass as bass
import concourse.tile as tile
from concourse import mybir
from concourse.masks import make_identity
from concourse.dve_ops import RECIPROCAL_APPROX_FAST, RECIP_APPROX_FAST_CONSTS

F32 = mybir.dt.float32
F32R = mybir.dt.float32r
ALU = mybir.AluOpType
ACTF = mybir.ActivationFunctionType

B, C, H, W = 32, 640, 32, 32
NH, KD, VD = 8, 64, 64
S = H * W            # 1024
P = 256              # key/value positions (16x16)
EPS = 1e-3
N_CORES = 8
BPC = B // N_CORES   # 4 batch items per core
NCH = C // 128       # 5 channel chunks


def _r(ap):
    return ap.bitcast(F32R)


def _fap(base, free_off, dims):
    """AP with base's partition dim and explicit free dims [[step, count],...]."""
    return bass.AP(tensor=base.tensor, offset=base.offset + free_off,
                   ap=[base.ap[0]] + dims)


def build_nc():
    nc = bacc.Bacc(None, target_bir_lowering=False, debug=False)

    din = {}
    def dt_in(name, shape):
        din[name] = nc.dram_tensor(name, shape, F32, kind="ExternalInput")
        return din[name]

    x4 = dt_in("x", [BPC, C, H, W])
    q_w = dt_in("q_w", [NH * KD, C])
    k_w = dt_in("k_w", [KD, C])
    v_w = dt_in("v_w", [VD, C])
    out_w = dt_in("out_w", [C, NH * VD])
    k_dw = dt_in("k_dw_w", [C, 1, 3, 3])
    v_dw = dt_in("v_dw_w", [C, 1, 3, 3])
    for p in ("in", "k", "v"):
        for s in ("gamma", "beta", "mean", "var"):
            dt_in(f"{p}_bn_{s}", [C])
    ls = dt_in("ls_gamma", [W])
    out4 = nc.dram_tensor("out", [BPC, C, H, W], F32, kind="ExternalOutput")
    KSTAGE = int(os.environ.get("KSTAGE", "99"))

    with tile.TileContext(nc) as tc, ExitStack() as ctx:
        wp = ctx.enter_context(tc.tile_pool(name="wp", bufs=1))
        stg = ctx.enter_context(tc.tile_pool(name="stg", bufs=2))
        # PSUM pools (bank-granular): mm 2 + lg 2 + op 4 = 8 banks
        mmp = ctx.enter_context(tc.tile_pool(name="mmp", bufs=2, space="PSUM"))
        lgp = ctx.enter_context(tc.tile_pool(name="lgp", bufs=2, space="PSUM"))
        opp = ctx.enter_context(tc.tile_pool(name="opp", bufs=4, space="PSUM"))
        # SBUF working pools
        xin = ctx.enter_context(tc.tile_pool(name="xin", bufs=2))
        xres = ctx.enter_context(tc.tile_pool(name="xres", bufs=2))
        xnp = ctx.enter_context(tc.tile_pool(name="xnp", bufs=NCH))
        xcp = ctx.enter_context(tc.tile_pool(name="xcp", bufs=1))
        qbp = ctx.enter_context(tc.tile_pool(name="qbp", bufs=2))
        ep = ctx.enter_context(tc.tile_pool(name="ep", bufs=3))
        dal = ctx.enter_context(tc.tile_pool(name="dal", bufs=1))
        rbcp = ctx.enter_context(tc.tile_pool(name="rbcp", bufs=2))
        orp = ctx.enter_context(tc.tile_pool(name="orp", bufs=4))
        osb = ctx.enter_context(tc.tile_pool(name="osb", bufs=2))
        kvp = ctx.enter_context(tc.tile_pool(name="kvp", bufs=2))
        drp = ctx.enter_context(tc.tile_pool(name="drp", bufs=4, space="DRAM"))

        # ---------------- setup: identity ----------------
        ident = wp.tile([128, 128], F32, tag="ident", name="ident")
        make_identity(nc, ident[:])

        def pe_transpose(dst_sbuf_ap, src_sbuf_ap, scale=1.0, rnd=False):
            """dst[f, p] = src[p, f] via PE; src [p, f] with p,f <= 128."""
            pdim = src_sbuf_ap.shape[0]
            fdim = src_sbuf_ap.free_size()
            tp = mmp.tile([128, 512], F32, tag="mm", name="tp")
            nc.tensor.transpose(tp[:fdim, :pdim], src_sbuf_ap,
                                ident[:pdim, :pdim])
            dst = _r(dst_sbuf_ap) if rnd else dst_sbuf_ap
            nc.scalar.activation(dst, tp[:fdim, :pdim], ACTF.Copy,
                                 scale=scale)

        # ---------------- setup: BN scale/shift ----------------
        eps_t = wp.tile([128, 1], F32, tag="eps", name="eps")
        nc.gpsimd.memset(eps_t[:], EPS)
        bnss = {}  # (prefix, ch) -> (scale [128,1], shift [128,1])
        for pfx in ("in", "k", "v"):
            for ch in range(NCH):
                g = stg.tile([128, 1], F32, tag="bnl0", name="bnl0")
                be = stg.tile([128, 1], F32, tag="bnl1", name="bnl1")
                m = stg.tile([128, 1], F32, tag="bnl2", name="bnl2")
                v = stg.tile([128, 1], F32, tag="bnl3", name="bnl3")
                cs = slice(128 * ch, 128 * (ch + 1))
                nc.sync.dma_start(out=g[:], in_=din[f"{pfx}_bn_gamma"][cs].unsqueeze(1))
                nc.sync.dma_start(out=be[:], in_=din[f"{pfx}_bn_beta"][cs].unsqueeze(1))
                nc.sync.dma_start(out=m[:], in_=din[f"{pfx}_bn_mean"][cs].unsqueeze(1))
                nc.sync.dma_start(out=v[:], in_=din[f"{pfx}_bn_var"][cs].unsqueeze(1))
                sc = wp.tile([128, 1], F32, tag=f"sc_{pfx}{ch}", name=f"sc_{pfx}{ch}")
                sh = wp.tile([128, 1], F32, tag=f"sh_{pfx}{ch}", name=f"sh_{pfx}{ch}")
                nc.scalar.activation(sc[:], v[:], ACTF.Sqrt, bias=eps_t[:])
                nc.vector.reciprocal(sc[:], sc[:])
                nc.vector.tensor_mul(sc[:], sc[:], g[:])
                nc.vector.tensor_mul(sh[:], m[:], sc[:])
                nc.vector.tensor_sub(sh[:], be[:], sh[:])
                bnss[(pfx, ch)] = (sc, sh)

        # ---------------- setup: transposed weights ----------------
        q_wT = [wp.tile([128, 512], F32, tag=f"qwT{j}", name=f"qwT{j}")
                for j in range(NCH)]
        for i in range(4):
            st = stg.tile([128, 640], F32, tag="wstage", name="wstage")
            nc.sync.dma_start(out=st[:], in_=q_w[128 * i:128 * (i + 1), :])
            for j in range(NCH):
                pe_transpose(q_wT[j][:, 128 * i:128 * (i + 1)],
                             st[:, 128 * j:128 * (j + 1)], rnd=True)

        out_wT = [wp.tile([128, 640], F32, tag=f"owT{j}", name=f"owT{j}")
                  for j in range(4)]
        for i in range(NCH):
            st = stg.tile([128, 512], F32, tag="wstage", name="wstage")
            nc.sync.dma_start(out=st[:], in_=out_w[128 * i:128 * (i + 1), :])
            for j in range(4):
                pe_transpose(out_wT[j][:, 128 * i:128 * (i + 1)],
                             st[:, 128 * j:128 * (j + 1)], rnd=True)

        kv_wT = {}
        for nm, wdr, scl in (("k", k_w, 0.125), ("v", v_w, 1.0)):
            st = stg.tile([64, 640], F32, tag="kvstage", name="kvstage")
            nc.sync.dma_start(out=st[:], in_=wdr[:, :])
            for j in range(NCH):
                wt = wp.tile([128, 64], F32, tag=f"{nm}wT{j}", name=f"{nm}wT{j}")
                pe_transpose(wt[:], st[:, 128 * j:128 * (j + 1)], scale=scl)
                kv_wT[(nm, j)] = wt

        # ---------------- setup: conv tap weights + consts ----------------
        wtap = {}
        kv_const = {}
        for nm, dwdr in (("k", k_dw), ("v", v_dw)):
            cps = mmp.tile([64, 512], F32, tag="mm", name="cps")
            for ch in range(NCH):
                dw = stg.tile([128, 9], F32, tag="dwl", name="dwl")
                nc.sync.dma_start(
                    out=dw[:],
                    in_=dwdr[128 * ch:128 * (ch + 1), 0, :, :].rearrange(
                        "c a b -> c (a b)"))
                sc, sh = bnss[(nm, ch)]
                s9 = stg.tile([128, 9], F32, tag="s9", name="s9")
                nc.vector.tensor_scalar_mul(s9[:], dw[:], sc[:])
                for t in range(9):
                    wtt = wp.tile([128, 64], F32, tag=f"wtap_{nm}{ch}_{t}",
                                  name=f"wtap_{nm}{ch}_{t}")
                    nc.vector.tensor_scalar_mul(_r(wtt[:]),
                                                kv_wT[(nm, ch)][:],
                                                s9[:, t:t + 1])
                    wtap[(nm, ch, t)] = wtt
                nc.tensor.matmul(cps[:64, 0:1], kv_wT[(nm, ch)][:], sh[:],
                                 start=(ch == 0), stop=(ch == NCH - 1))
            cst = wp.tile([64, 1], F32, tag=f"const_{nm}", name=f"const_{nm}")
            nc.scalar.activation(cst[:], cps[:64, 0:1], ACTF.Copy)
            kv_const[nm] = cst

        # ---------------- setup: zero/one consts ----------------
        zeros16 = wp.tile([128, 16], F32, tag="zeros16", name="zeros16")
        nc.gpsimd.memset(zeros16[:], 0.0)
        ones1 = wp.tile([128, 1], F32, tag="ones1", name="ones1")
        nc.gpsimd.memset(ones1[:], 1.0)

        # ---------------- setup: layer-scale mask ----------------
        lsmask = wp.tile([128, 1024], F32, tag="lsmask", name="lsmask")
        ls_b = bass.AP(tensor=ls, offset=0, ap=[[0, 128], [1, 32]])
        for rr in range(32):
            nc.sync.dma_start(out=lsmask[:, 32 * rr:32 * (rr + 1)], in_=ls_b)

        # ================= per batch item =================
        im2col_eng = [nc.vector, nc.gpsimd, nc.scalar]
        for b in range(BPC):
            # ---- load x, BN into flat xn buffer ----
            xns = []
            for ch in range(NCH):
                xt = xin.tile([128, 1024], F32, tag="xin", name="xin")
                nc.sync.dma_start(
                    out=xt[:],
                    in_=x4[b, 128 * ch:128 * (ch + 1), :, :].rearrange(
                        "c h w -> c (h w)"))
                xn = xnp.tile([128, 1024], F32, tag="xn", name="xn")
                sc, sh = bnss[("in", ch)]
                nc.gpsimd.tensor_scalar(
                    out=_r(xn[:]), in0=xt[:],
                    scalar1=sc[:], scalar2=sh[:], op0=ALU.mult, op1=ALU.add)
                xns.append(xn)

            # ---- q projection -> qbuf [s%128, c*8 + t] (c-major) ----
            qbuf = qbp.tile([128, 4096], F32, tag="qbuf", name="qbuf")
            for t in range(8):
                qp = mmp.tile([128, 512], F32, tag="mm", name="qp")
                for ch in range(NCH):
                    lhsT = xns[ch][:, 128 * t:128 * (t + 1)]
                    nc.tensor.matmul(qp[:], _r(lhsT), _r(q_wT[ch][:]),
                                     start=(ch == 0), stop=(ch == NCH - 1))
                nc.vector.tensor_copy(_r(_fap(qbuf[:], t, [[8, 512]])), qp[:])

            if KSTAGE == 1:
                nc.sync.dma_start(
                    out=out4[b, 0:128, :, :].rearrange("c h w -> c (h w)"),
                    in_=qbuf[:, 0:1024])
                continue
            # ---- im2col + dw-conv + BN + 1x1 proj for k and v ----
            kfp = mmp.tile([64, 256], F32, tag="mm", name="kfp")
            vfp = mmp.tile([64, 256], F32, tag="mm", name="vfp")
            for ch in range(NCH):
                xc = xcp.tile([128, 9 * 256], F32, tag="xcol", name="xcol")
                xnv = xns[ch][:].rearrange("p (a b) -> p a b", a=32)
                for t in range(9):
                    dy, dx = t // 3, t % 3
                    oh0 = 1 if dy == 0 else 0
                    ow0 = 1 if dx == 0 else 0
                    if oh0:
                        nc.vector.tensor_copy(_r(xc[:, 256 * t:256 * t + 16]),
                                              zeros16[:])
                    if ow0:
                        nc.vector.tensor_copy(
                            _r(_fap(xc[:], 256 * t, [[16, 16], [1, 1]])),
                            zeros16[:])
                    r0 = 2 * oh0 + dy - 1
                    c0 = 2 * ow0 + dx - 1
                    srcap = xnv[:, r0:r0 + 2 * (16 - oh0) - 1:2,
                                c0:c0 + 2 * (16 - ow0) - 1:2]
                    dst2 = _r(_fap(xc[:], 256 * t + 16 * oh0 + ow0,
                                   [[16, 16 - oh0], [1, 16 - ow0]]))
                    if t % 3 == 2:
                        nc.scalar.activation(dst2, srcap, ACTF.Copy)
                    else:
                        im2col_eng[t % 3].tensor_copy(dst2, srcap)
                for t in range(9):
                    first = (ch == 0 and t == 0)
                    last = (ch == NCH - 1 and t == 8)
                    xslice = xc[:, 256 * t:256 * (t + 1)]
                    nc.tensor.matmul(
                        kfp[:], _r(wtap[("k", ch, t)][:]), _r(xslice),
                        start=first, stop=last)
                    nc.tensor.matmul(
                        vfp[:], _r(wtap[("v", ch, t)][:]), _r(xslice),
                        start=first, stop=last)
            # kf duplicated into both halves (base-partition match for logits)
            kfdup = kvp.tile([128, 256], F32, tag="f_k", name="f_k")
            nc.vector.tensor_scalar_add(_r(kfdup[0:64, :]), kfp[:],
                                        kv_const["k"][:])
            nc.vector.tensor_scalar_add(_r(kfdup[64:128, :]), kfp[:],
                                        kv_const["k"][:])
            vf = kvp.tile([64, 256], F32, tag="f_v", name="f_v")
            nc.vector.tensor_scalar_add(vf[:], vfp[:],
                                        kv_const["v"][:])

            # V' = vf^T with ones column: 2 tiles [128, 65]
            vT = []
            for pt in range(2):
                vpt = kvp.tile([128, 65], F32, tag=f"vT{pt}", name=f"vT{pt}")
                pe_transpose(vpt[:, 0:64], vf[:, 128 * pt:128 * (pt + 1)],
                             rnd=True)
                nc.vector.tensor_copy(_r(vpt[:, 64:65]), ones1[:])
                vT.append(vpt)

            if KSTAGE == 2:
                nc.sync.dma_start(
                    out=out4[b, 0:128, 0:8, :].rearrange("c h w -> c (h w)"),
                    in_=kfdup[:, :])
                nc.sync.dma_start(
                    out=out4[b, 128:256, 0:2, :].rearrange("c h w -> c (h w)"),
                    in_=vT[0][:, 0:64])
                continue
            o_resh = [orp.tile([128, 1024], F32, tag="oresh", name="oresh")
                      for _ in range(4)]

            # ---- attention heads (pairs share a reciprocal) ----
            for pair in range(4):
                dall = dal.tile([33, 1024], F32, tag="dall", name="dall")
                ops_pair = []
                for n in (2 * pair, 2 * pair + 1):
                    E = [ep.tile([128, 1024], F32, tag="E", name="E")
                         for _ in range(2)]
                    for pt in range(2):
                        for par in range(2):
                            lg = lgp.tile([128, 512], F32, tag="lg", name="lg")
                            rhs = qbuf[64 * par:64 * (par + 1),
                                       512 * n:512 * (n + 1)]
                            nc.tensor.matmul(
                                lg[:],
                                _r(kfdup[64 * par:64 * (par + 1),
                                         128 * pt:128 * (pt + 1)]),
                                _r(rhs), start=True, stop=True)
                            nc.scalar.activation(
                                _r(E[pt][:, 512 * par:512 * (par + 1)]), lg[:],
                                ACTF.Exp)
                    o_ps = []
                    for par in range(2):
                        op_t = opp.tile([65, 512], F32, tag="op", name="op")
                        for pt in range(2):
                            nc.tensor.matmul(
                                op_t[:], _r(vT[pt][:]),
                                _r(E[pt][:, 512 * par:512 * (par + 1)]),
                                start=(pt == 0), stop=(pt == 1))
                        nc.scalar.activation(
                            dall[32 * (n % 2):32 * (n % 2) + 1,
                                 512 * par:512 * (par + 1)],
                            op_t[64:65, :], ACTF.Copy)
                        o_ps.append(op_t)
                    ops_pair.append((n, o_ps))

                rec = dal.tile([33, 1024], F32, tag="rec", name="rec")
                nc.vector._custom_dve(
                    RECIPROCAL_APPROX_FAST, out=rec[:], in0=dall[:],
                    s0=RECIP_APPROX_FAST_CONSTS["s0"],
                    s1=RECIP_APPROX_FAST_CONSTS["s1"],
                    imm2=RECIP_APPROX_FAST_CONSTS["imm2"])

                for n, o_ps in ops_pair:
                    dsc = drp.tile([1, 1024], F32, tag="dscr", name="dscr")
                    nc.sync.dma_start(
                        out=dsc[:], in_=rec[32 * (n % 2):32 * (n % 2) + 1, :])
                    rbc = rbcp.tile([64, 1024], F32, tag="rbc", name="rbc")
                    nc.sync.dma_start(
                        out=rbc[:],
                        in_=bass.AP(tensor=dsc.tensor, offset=dsc.offset,
                                    ap=[[0, 64], [1, 1024]]))
                    dst = o_resh[n // 2]
                    for par in range(2):
                        # scatter: col = 16*c + 2*t + par, iteration c-major
                        out_ap = _fap(dst[64 * (n % 2):64 * (n % 2) + 64], par,
                                      [[16, 64], [2, 8]])
                        nc.vector.scalar_tensor_tensor(
                            out=_r(out_ap), in0=o_ps[par][0:64, :], scalar=1.0,
                            in1=rbc[:, 512 * par:512 * (par + 1)],
                            op0=ALU.mult, op1=ALU.mult)

            if KSTAGE == 3:
                for c2 in range(4):
                    nc.sync.dma_start(
                        out=out4[b, 128 * c2:128 * (c2 + 1), :, :].rearrange(
                            "c h w -> c (h w)"),
                        in_=o_resh[c2][:, :])
                continue
            # ---- output projection + layer scale + residual ----
            for ch in range(NCH):
                xr = xres.tile([128, 1024], F32, tag="xres", name="xres")
                nc.sync.dma_start(
                    out=xr[:],
                    in_=x4[b, 128 * ch:128 * (ch + 1), :, :].rearrange(
                        "c h w -> c (h w)"))
                ot = osb.tile([128, 1024], F32, tag="outsb", name="outsb")
                for shalf in range(2):
                    po = mmp.tile([128, 512], F32, tag="mm", name="po")
                    for nv in range(4):
                        nc.tensor.matmul(
                            po[:],
                            _r(out_wT[nv][:, 128 * ch:128 * (ch + 1)]),
                            _r(o_resh[nv][:, 512 * shalf:512 * (shalf + 1)]),
                            start=(nv == 0), stop=(nv == 3))
                    sl = slice(512 * shalf, 512 * (shalf + 1))
                    nc.vector.scalar_tensor_tensor(
                        out=ot[:, sl], in0=po[:], scalar=1.0,
                        in1=lsmask[:, sl], op0=ALU.mult, op1=ALU.mult)
                    nc.gpsimd.tensor_tensor(
                        out=ot[:, sl], in0=ot[:, sl], in1=xr[:, sl], op=ALU.add)
                nc.sync.dma_start(
                    out=out4[b, 128 * ch:128 * (ch + 1), :, :].rearrange(
                        "c h w -> c (h w)"),
                    in_=ot[:])

    nc.finalize()
    return nc


_NC_CACHE = None


def kernel(**inputs):
    global _NC_CACHE
    from concourse.bass_utils import run_bass_kernel_spmd

    if _NC_CACHE is None:
        _NC_CACHE = build_nc()
    nc = _NC_CACHE

    x = np.ascontiguousarray(np.asarray(inputs["x"], dtype=np.float32))
    wnames = ["q_w", "k_w", "v_w", "out_w", "k_dw_w", "v_dw_w", "ls_gamma"] + \
        [f"{p}_bn_{s}" for p in ("in", "k", "v")
         for s in ("gamma", "beta", "mean", "var")]
    base = {n: np.ascontiguousarray(np.asarray(inputs[n], dtype=np.float32))
            for n in wnames}
    in_maps = []
    for c in range(N_CORES):
        m = dict(base)
        m["x"] = x[c * BPC:(c + 1) * BPC]
        in_maps.append(m)

    res = run_bass_kernel_spmd(nc, in_maps, core_ids=list(range(N_CORES)))
    out = np.concatenate([res.results[c]["out"] for c in range(N_CORES)], axis=0)
    return out.astype(np.float32)


# revision 17
# speedup vs baseline: 1.0857x; 1.0271x over previous
"""Trainium2 Bass kernel for nn_MultiHeadSelfAttentionBlock.

Strategy (data-parallel over batch, B=32 -> 4 per core on 8 cores):
  - BN folded to per-channel scale/shift, written into a zero-padded [C,34,34]
    SBUF buffer (borders = conv padding).
  - q 1x1 conv as 40 accumulating matmuls; results stored c-major:
    qbuf[s%128, c*8 + t] so each head's Q^T tile is one contiguous slice
    (the torch .view head-split bug resolves to l = 16*c + 2*t + par, k = s_lo).
  - depthwise 3x3/s2 conv + BN + 1x1 proj: im2col tap windows staged once per
    channel chunk (shared by k and v), then 9 taps x 5 chunks of accumulating
    matmuls with weights W_tapT[c,kd] = wT[c,kd]*bnscale[c]*dw[c,tap]
    (k-side prescaled by 1/sqrt(64)); BN shift folded into a constant.
    k and v share one PSUM tile via PE column groups (0,0) / (0,64).
  - logits computed transposed [p, l] (lhsT = kf duplicated into both
    partition halves so K=64 matmul pairs pack into PE row groups 0/64);
    softmax denominator comes free as row 64 of the o-matmul by appending a
    ones column to V^T.
  - o = V'^T @ exp(logitsT) in PSUM [65, l]; denominator rows extracted by
    ACT into partitions 0/32, reciprocal via ACT ln -> exp(-x), broadcast
    across partitions via a DRAM bounce, and the normalize multiply fused
    into the PSUM->SBUF move (DVE STT), writing o_resh[(n,vd), s] directly
    via a scatter AP.
  - output proj as accumulating matmuls; layer-scale applied via a
    precomputed [128,1024] mask during the PSUM->SBUF move; residual added on
    GPSIMD; fp32 everywhere, matmuls use float32r (fast fp32 weight path).
"""

from contextlib import ExitStack

import os

import numpy as np

import concourse.bacc as bacc
import concourse.bass as bass
import concourse.tile as tile
from concourse import mybir
from concourse.masks import make_identity
from concourse.dve_ops import RECIPROCAL_APPROX_FAST, RECIP_APPROX_FAST_CONSTS

F32 = mybir.dt.float32
F32R = mybir.dt.float32r
ALU = mybir.AluOpType
ACTF = mybir.ActivationFunctionType

B, C, H, W = 32, 640, 32, 32
NH, KD, VD = 8, 64, 64
S = H * W            # 1024
P = 256              # key/value positions (16x16)
EPS = 1e-3
N_CORES = 8
BPC = B // N_CORES   # 4 batch items per core
NCH = C // 128       # 5 channel chunks


def _r(ap):
    return ap.bitcast(F32R)


def _fap(base, free_off, dims):
    """AP with base's partition dim and explicit free dims [[step, count],...]."""
    return bass.AP(tensor=base.tensor, offset=base.offset + free_off,
                   ap=[base.ap[0]] + dims)


def build_nc():
    nc = bacc.Bacc(None, target_bir_lowering=False, debug=False)

    din = {}
    def dt_in(name, shape):
        din[name] = nc.dram_tensor(name, shape, F32, kind="ExternalInput")
        return din[name]

    x4 = dt_in("x", [BPC, C, H, W])
    q_w = dt_in("q_w", [NH * KD, C])
    k_w = dt_in("k_w", [KD, C])
    v_w = dt_in("v_w", [VD, C])
    out_w = dt_in("out_w", [C, NH * VD])
    k_dw = dt_in("k_dw_w", [C, 1, 3, 3])
    v_dw = dt_in("v_dw_w", [C, 1, 3, 3])
    for p in ("in", "k", "v"):
        for s in ("gamma", "beta", "mean", "var"):
            dt_in(f"{p}_bn_{s}", [C])
    ls = dt_in("ls_gamma", [W])
    out4 = nc.dram_tensor("out", [BPC, C, H, W], F32, kind="ExternalOutput")
    KSTAGE = int(os.environ.get("KSTAGE", "99"))

    with tile.TileContext(nc) as tc, ExitStack() as ctx:
        wp = ctx.enter_context(tc.tile_pool(name="wp", bufs=1))
        stg = ctx.enter_context(tc.tile_pool(name="stg", bufs=2))
        # PSUM pools (bank-granular): mm 2 + lg 2 + op 4 = 8 banks
        mmp = ctx.enter_context(tc.tile_pool(name="mmp", bufs=2, space="PSUM"))
        lgp = ctx.enter_context(tc.tile_pool(name="lgp", bufs=2, space="PSUM"))
        opp = ctx.enter_context(tc.tile_pool(name="opp", bufs=4, space="PSUM"))
        # SBUF working pools
        xin = ctx.enter_context(tc.tile_pool(name="xin", bufs=2))
        xres = ctx.enter_context(tc.tile_pool(name="xres", bufs=2))
        xnp = ctx.enter_context(tc.tile_pool(name="xnp", bufs=NCH))
        xcp = ctx.enter_context(tc.tile_pool(name="xcp", bufs=1))
        qbp = ctx.enter_context(tc.tile_pool(name="qbp", bufs=2))
        ep = ctx.enter_context(tc.tile_pool(name="ep", bufs=3))
        dal = ctx.enter_context(tc.tile_pool(name="dal", bufs=1))
        rbcp = ctx.enter_context(tc.tile_pool(name="rbcp", bufs=2))
        orp = ctx.enter_context(tc.tile_pool(name="orp", bufs=4))
        osb = ctx.enter_context(tc.tile_pool(name="osb", bufs=2))
        kvp = ctx.enter_context(tc.tile_pool(name="kvp", bufs=2))
        drp = ctx.enter_context(tc.tile_pool(name="drp", bufs=4, space="DRAM"))

        # ---------------- setup: identity ----------------
        ident = wp.tile([128, 128], F32, tag="ident", name="ident")
        make_identity(nc, ident[:])

        def pe_transpose(dst_sbuf_ap, src_sbuf_ap, scale=1.0, rnd=False):
            """dst[f, p] = src[p, f] via PE; src [p, f] with p,f <= 128."""
            pdim = src_sbuf_ap.shape[0]
            fdim = src_sbuf_ap.free_size()
            tp = mmp.tile([128, 512], F32, tag="mm", name="tp")
            nc.tensor.transpose(tp[:fdim, :pdim], src_sbuf_ap,
                                ident[:pdim, :pdim])
            dst = _r(dst_sbuf_ap) if rnd else dst_sbuf_ap
            nc.scalar.activation(dst, tp[:fdim, :pdim], ACTF.Copy,
                                 scale=scale)

        # ---------------- setup: BN scale/shift ----------------
        eps_t = wp.tile([128, 1], F32, tag="eps", name="eps")
        nc.gpsimd.memset(eps_t[:], EPS)
        bnss = {}  # (prefix, ch) -> (scale [128,1], shift [128,1])
        for pfx in ("in", "k", "v"):
            for ch in range(NCH):
                g = stg.tile([128, 1], F32, tag="bnl0", name="bnl0")
                be = stg.tile([128, 1], F32, tag="bnl1", name="bnl1")
                m = stg.tile([128, 1], F32, tag="bnl2", name="bnl2")
                v = stg.tile([128, 1], F32, tag="bnl3", name="bnl3")
                cs = slice(128 * ch, 128 * (ch + 1))
                nc.sync.dma_start(out=g[:], in_=din[f"{pfx}_bn_gamma"][cs].unsqueeze(1))
                nc.sync.dma_start(out=be[:], in_=din[f"{pfx}_bn_beta"][cs].unsqueeze(1))
                nc.sync.dma_start(out=m[:], in_=din[f"{pfx}_bn_mean"][cs].unsqueeze(1))
                nc.sync.dma_start(out=v[:], in_=din[f"{pfx}_bn_var"][cs].unsqueeze(1))
                sc = wp.tile([128, 1], F32, tag=f"sc_{pfx}{ch}", name=f"sc_{pfx}{ch}")
                sh = wp.tile([128, 1], F32, tag=f"sh_{pfx}{ch}", name=f"sh_{pfx}{ch}")
                nc.scalar.activation(sc[:], v[:], ACTF.Sqrt, bias=eps_t[:])
                nc.vector.reciprocal(sc[:], sc[:])
                nc.vector.tensor_mul(sc[:], sc[:], g[:])
                nc.vector.tensor_mul(sh[:], m[:], sc[:])
                nc.vector.tensor_sub(sh[:], be[:], sh[:])
                bnss[(pfx, ch)] = (sc, sh)

        # ---------------- setup: transposed weights ----------------
        q_wT = [wp.tile([128, 512], F32, tag=f"qwT{j}", name=f"qwT{j}")
                for j in range(NCH)]
        for i in range(4):
            st = stg.tile([128, 640], F32, tag="wstage", name="wstage")
            nc.sync.dma_start(out=st[:], in_=q_w[128 * i:128 * (i + 1), :])
            for j in range(NCH):
                pe_transpose(q_wT[j][:, 128 * i:128 * (i + 1)],
                             st[:, 128 * j:128 * (j + 1)], rnd=True)

        out_wT = [wp.tile([128, 640], F32, tag=f"owT{j}", name=f"owT{j}")
                  for j in range(4)]
        for i in range(NCH):
            st = stg.tile([128, 512], F32, tag="wstage", name="wstage")
            nc.sync.dma_start(out=st[:], in_=out_w[128 * i:128 * (i + 1), :])
            for j in range(4):
                pe_transpose(out_wT[j][:, 128 * i:128 * (i + 1)],
                             st[:, 128 * j:128 * (j + 1)], rnd=True)

        kv_wT = {}
        for nm, wdr, scl in (("k", k_w, 0.125), ("v", v_w, 1.0)):
            st = stg.tile([64, 640], F32, tag="kvstage", name="kvstage")
            nc.sync.dma_start(out=st[:], in_=wdr[:, :])
            for j in range(NCH):
                wt = wp.tile([128, 64], F32, tag=f"{nm}wT{j}", name=f"{nm}wT{j}")
                pe_transpose(wt[:], st[:, 128 * j:128 * (j + 1)], scale=scl)
                kv_wT[(nm, j)] = wt

        # ---------------- setup: conv tap weights + consts ----------------
        wtap = {}
        kv_const = {}
        for nm, dwdr in (("k", k_dw), ("v", v_dw)):
            cps = mmp.tile([64, 512], F32, tag="mm", name="cps")
            for ch in range(NCH):
                dw = stg.tile([128, 9], F32, tag="dwl", name="dwl")
                nc.sync.dma_start(
                    out=dw[:],
                    in_=dwdr[128 * ch:128 * (ch + 1), 0, :, :].rearrange(
                        "c a b -> c (a b)"))
                sc, sh = bnss[(nm, ch)]
                s9 = stg.tile([128, 9], F32, tag="s9", name="s9")
                nc.vector.tensor_scalar_mul(s9[:], dw[:], sc[:])
                for t in range(9):
                    wtt = wp.tile([128, 64], F32, tag=f"wtap_{nm}{ch}_{t}",
                                  name=f"wtap_{nm}{ch}_{t}")
                    nc.vector.tensor_scalar_mul(_r(wtt[:]),
                                                kv_wT[(nm, ch)][:],
                                                s9[:, t:t + 1])
                    wtap[(nm, ch, t)] = wtt
                nc.tensor.matmul(cps[:64, 0:1], kv_wT[(nm, ch)][:], sh[:],
                                 start=(ch == 0), stop=(ch == NCH - 1))
            cst = wp.tile([64, 1], F32, tag=f"const_{nm}", name=f"const_{nm}")
            nc.scalar.activation(cst[:], cps[:64, 0:1], ACTF.Copy)
            kv_const[nm] = cst

        # ---------------- setup: zero/one consts ----------------
        zeros16 = wp.tile([128, 16], F32, tag="zeros16", name="zeros16")
        nc.gpsimd.memset(zeros16[:], 0.0)
        ones1 = wp.tile([128, 1], F32, tag="ones1", name="ones1")
        nc.gpsimd.memset(ones1[:], 1.0)

        # ---------------- setup: layer-scale mask ----------------
        lsmask = wp.tile([128, 1024], F32, tag="lsmask", name="lsmask")
        ls_b = bass.AP(tensor=ls, offset=0, ap=[[0, 128], [1, 32]])
        for rr in range(32):
            nc.sync.dma_start(out=lsmask[:, 32 * rr:32 * (rr + 1)], in_=ls_b)

        # ================= per batch item =================
        im2col_eng = [nc.vector, nc.gpsimd, nc.scalar]
        for b in range(BPC):
            # ---- load x, BN into flat xn buffer ----
            xns = []
            for ch in range(NCH):
                xt = xin.tile([128, 1024], F32, tag="xin", name="xin")
                nc.sync.dma_start(
                    out=xt[:],
                    in_=x4[b, 128 * ch:128 * (ch + 1), :, :].rearrange(
                        "c h w -> c (h w)"))
                xn = xnp.tile([128, 1024], F32, tag="xn", name="xn")
                sc, sh = bnss[("in", ch)]
                nc.gpsimd.tensor_scalar(
                    out=_r(xn[:]), in0=xt[:],
                    scalar1=sc[:], scalar2=sh[:], op0=ALU.mult, op1=ALU.add)
                xns.append(xn)

            # ---- q projection -> qbuf [s%128, c*8 + t] (c-major) ----
            qbuf = qbp.tile([128, 4096], F32, tag="qbuf", name="qbuf")
            for t in range(8):
                qp = mmp.tile([128, 512], F32, tag="mm", name="qp")
                for ch in range(NCH):
                    lhsT = xns[ch][:, 128 * t:128 * (t + 1)]
                    nc.tensor.matmul(qp[:], _r(lhsT), _r(q_wT[ch][:]),
                                     start=(ch == 0), stop=(ch == NCH - 1))
                nc.vector.tensor_copy(_r(_fap(qbuf[:], t, [[8, 512]])), qp[:])

            if KSTAGE == 1:
                nc.sync.dma_start(
                    out=out4[b, 0:128, :, :].rearrange("c h w -> c (h w)"),
                    in_=qbuf[:, 0:1024])
                continue
            # ---- im2col + dw-conv + BN + 1x1 proj for k and v ----
            kfp = mmp.tile([64, 256], F32, tag="mm", name="kfp")
            vfp = mmp.tile([64, 256], F32, tag="mm", name="vfp")
            for ch in range(NCH):
                xc = xcp.tile([128, 9 * 256], F32, tag="xcol", name="xcol")
                xnv = xns[ch][:].rearrange("p (a b) -> p a b", a=32)
                for t in range(9):
                    dy, dx = t // 3, t % 3
                    oh0 = 1 if dy == 0 else 0
                    ow0 = 1 if dx == 0 else 0
                    if oh0:
                        nc.vector.tensor_copy(_r(xc[:, 256 * t:256 * t + 16]),
                                              zeros16[:])
                    if ow0:
                        nc.vector.tensor_copy(
                            _r(_fap(xc[:], 256 * t, [[16, 16], [1, 1]])),
                            zeros16[:])
                    r0 = 2 * oh0 + dy - 1
                    c0 = 2 * ow0 + dx - 1
                    srcap = xnv[:, r0:r0 + 2 * (16 - oh0) - 1:2,
                                c0:c0 + 2 * (16 - ow0) - 1:2]
                    dst2 = _r(_fap(xc[:], 256 * t + 16 * oh0 + ow0,
                                   [[16, 16 - oh0], [1, 16 - ow0]]))
                    if t % 3 == 2:
                        nc.scalar.activation(dst2, srcap, ACTF.Copy)
                    else:
                        im2col_eng[t % 3].tensor_copy(dst2, srcap)
                for t in range(9):
                    first = (ch == 0 and t == 0)
                    last = (ch == NCH - 1 and t == 8)
                    xslice = xc[:, 256 * t:256 * (t + 1)]
                    nc.tensor.matmul(
                        kfp[:], _r(wtap[("k", ch, t)][:]), _r(xslice),
                        start=first, stop=last)
                    nc.tensor.matmul(
                        vfp[:], _r(wtap[("v", ch, t)][:]), _r(xslice),
                        start=first, stop=last)
            # kf duplicated into both halves (base-partition match for logits)
            kfdup = kvp.tile([128, 256], F32, tag="f_k", name="f_k")
            nc.vector.tensor_scalar_add(_r(kfdup[0:64, :]), kfp[:],
                                        kv_const["k"][:])
            nc.vector.tensor_scalar_add(_r(kfdup[64:128, :]), kfp[:],
                                        kv_const["k"][:])
            vf = kvp.tile([64, 256], F32, tag="f_v", name="f_v")
            nc.vector.tensor_scalar_add(vf[:], vfp[:],
                                        kv_const["v"][:])

            # V' = vf^T with ones column: 2 tiles [128, 65]
            vT = []
            for pt in range(2):
                vpt = kvp.tile([128, 65], F32, tag=f"vT{pt}", name=f"vT{pt}")
                pe_transpose(vpt[:, 0:64], vf[:, 128 * pt:128 * (pt + 1)],
                             rnd=True)
                nc.vector.tensor_copy(_r(vpt[:, 64:65]), ones1[:])
                vT.append(vpt)

            if KSTAGE == 2:
                nc.sync.dma_start(
                    out=out4[b, 0:128, 0:8, :].rearrange("c h w -> c (h w)"),
                    in_=kfdup[:, :])
                nc.sync.dma_start(
                    out=out4[b, 128:256, 0:2, :].rearrange("c h w -> c (h w)"),
                    in_=vT[0][:, 0:64])
                continue
            o_resh = [orp.tile([128, 1024], F32, tag="oresh", name="oresh")
                      for _ in range(4)]

            # ---- attention heads (pairs share a reciprocal) ----
            for pair in range(4):
                dall = dal.tile([33, 1024], F32, tag="dall", name="dall")
                ops_pair = []
                for n in (2 * pair, 2 * pair + 1):
                    E = [ep.tile([128, 1024], F32, tag="E", name="E")
                         for _ in range(2)]
                    for pt in range(2):
                        for par in range(2):
                            lg = lgp.tile([128, 512], F32, tag="lg", name="lg")
                            rhs = qbuf[64 * par:64 * (par + 1),
                                       512 * n:512 * (n + 1)]
                            nc.tensor.matmul(
                                lg[:],
                                _r(kfdup[64 * par:64 * (par + 1),
                                         128 * pt:128 * (pt + 1)]),
                                _r(rhs), start=True, stop=True)
                            nc.scalar.activation(
                                _r(E[pt][:, 512 * par:512 * (par + 1)]), lg[:],
                                ACTF.Exp)
                    o_ps = []
                    for par in range(2):
                        op_t = opp.tile([65, 512], F32, tag="op", name="op")
                        for pt in range(2):
                            nc.tensor.matmul(
                                op_t[:], _r(vT[pt][:]),
                                _r(E[pt][:, 512 * par:512 * (par + 1)]),
                                start=(pt == 0), stop=(pt == 1))
                        nc.vector.tensor_copy(
                            dall[32 * (n % 2):32 * (n % 2) + 1,
                                 512 * par:512 * (par + 1)],
                            op_t[64:65, :])
                        o_ps.append(op_t)
                    ops_pair.append((n, o_ps))

                rec = dal.tile([33, 1024], F32, tag="rec", name="rec")
                nc.vector._custom_dve(
                    RECIPROCAL_APPROX_FAST, out=rec[:], in0=dall[:],
                    s0=RECIP_APPROX_FAST_CONSTS["s0"],
                    s1=RECIP_APPROX_FAST_CONSTS["s1"],
                    imm2=RECIP_APPROX_FAST_CONSTS["imm2"])

                for n, o_ps in ops_pair:
                    dsc = drp.tile([1, 1024], F32, tag="dscr", name="dscr")
                    nc.sync.dma_start(
                        out=dsc[:], in_=rec[32 * (n % 2):32 * (n % 2) + 1, :])
                    rbc = rbcp.tile([64, 1024], F32, tag="rbc", name="rbc")
                    nc.sync.dma_start(
                        out=rbc[:],
                        in_=bass.AP(tensor=dsc.tensor, offset=dsc.offset,
                                    ap=[[0, 64], [1, 1024]]))
                    dst = o_resh[n // 2]
                    for par in range(2):
                        # scatter: col = 16*c + 2*t + par, iteration c-major
                        out_ap = _fap(dst[64 * (n % 2):64 * (n % 2) + 64], par,
                                      [[16, 64], [2, 8]])
                        nc.vector.scalar_tensor_tensor(
                            out=_r(out_ap), in0=o_ps[par][0:64, :], scalar=1.0,
                            in1=rbc[:, 512 * par:512 * (par + 1)],
                            op0=ALU.mult, op1=ALU.mult)

            if KSTAGE == 3:
                for c2 in range(4):
                    nc.sync.dma_start(
                        out=out4[b, 128 * c2:128 * (c2 + 1), :, :].rearrange(
                            "c h w -> c (h w)"),
                        in_=o_resh[c2][:, :])
                continue
            # ---- output projection + layer scale + residual ----
            for ch in range(NCH):
                xr = xres.tile([128, 1024], F32, tag="xres", name="xres")
                nc.sync.dma_start(
                    out=xr[:],
                    in_=x4[b, 128 * ch:128 * (ch + 1), :, :].rearrange(
                        "c h w -> c (h w)"))
                ot = osb.tile([128, 1024], F32, tag="outsb", name="outsb")
                for shalf in range(2):
                    po = mmp.tile([128, 512], F32, tag="mm", name="po")
                    for nv in range(4):
                        nc.tensor.matmul(
                            po[:],
                            _r(out_wT[nv][:, 128 * ch:128 * (ch + 1)]),
                            _r(o_resh[nv][:, 512 * shalf:512 * (shalf + 1)]),
                            start=(nv == 0), stop=(nv == 3))
                    sl = slice(512 * shalf, 512 * (shalf + 1))
                    nc.vector.scalar_tensor_tensor(
                        out=ot[:, sl], in0=po[:], scalar=1.0,
                        in1=lsmask[:, sl], op0=ALU.mult, op1=ALU.mult)
                    nc.gpsimd.tensor_tensor(
                        out=ot[:, sl], in0=ot[:, sl], in1=xr[:, sl], op=ALU.add)
                nc.sync.dma_start(
                    out=out4[b, 128 * ch:128 * (ch + 1), :, :].rearrange(
                        "c h w -> c (h w)"),
                    in_=ot[:])

    nc.finalize()
    return nc


_NC_CACHE = None


def kernel(**inputs):
    global _NC_CACHE
    from concourse.bass_utils import run_bass_kernel_spmd

    if _NC_CACHE is None:
        _NC_CACHE = build_nc()
    nc = _NC_CACHE

    x = np.ascontiguousarray(np.asarray(inputs["x"], dtype=np.float32))
    wnames = ["q_w", "k_w", "v_w", "out_w", "k_dw_w", "v_dw_w", "ls_gamma"] + \
        [f"{p}_bn_{s}" for p in ("in", "k", "v")
         for s in ("gamma", "beta", "mean", "var")]
    base = {n: np.ascontiguousarray(np.asarray(inputs[n], dtype=np.float32))
            for n in wnames}
    in_maps = []
    for c in range(N_CORES):
        m = dict(base)
        m["x"] = x[c * BPC:(c + 1) * BPC]
        in_maps.append(m)

    res = run_bass_kernel_spmd(nc, in_maps, core_ids=list(range(N_CORES)))
    out = np.concatenate([res.results[c]["out"] for c in range(N_CORES)], axis=0)
    return out.astype(np.float32)
